# revision 17
# baseline (speedup 1.0000x reference)
import sys

import numpy as np

sys.path.insert(0, "/opt/trn_rl_repo")

import concourse.bacc as bacc
import concourse.tile as tile
from concourse import mybir
from concourse.bass_utils import run_bass_kernel_spmd
from concourse.masks import make_identity

BS, T, IN, STATE, OUT = 256, 128, 128, 1024, 1024
NCORES = 8
BSH = BS // NCORES  # 32 batch rows per core
NCH = STATE // 128  # 8 state chunks of 128
TB = 16             # timesteps per ext block
NTB = T // TB       # 8
RING = 3            # ext ring depth (blocks resident)
NG = 4              # PE column-tile groups for the recurrence matmul
GW = STATE // NG    # 256 output state cols per group

TRACE = False
BG_PER_STEP = 2

LAST_EXEC_NS = None
LAST_RESULTS = None
_DONE = object()

F32 = mybir.dt.float32
BF16 = mybir.dt.bfloat16
RELU = mybir.ActivationFunctionType.Relu


def _build(tc, x_d, w_in_d, b_in_d, w_rec_d, b_rec_d, w_out_d, b_out_d, out_d):
    nc = tc.nc

    with (
        tc.tile_pool(name="persist", bufs=1) as persist,
        tc.tile_pool(name="extp", bufs=RING) as extp,
        tc.tile_pool(name="nat", bufs=2) as nat,
        tc.tile_pool(name="small", bufs=2) as small,
        tc.tile_pool(name="xts_p", bufs=2) as xts_p,
        tc.tile_pool(name="st", bufs=2) as stp,
        tc.tile_pool(name="zsb", bufs=2) as zsbp,
        tc.tile_pool(name="ps_z", bufs=2, space="PSUM") as ps_z,
        tc.tile_pool(name="ps_dum", bufs=1, space="PSUM") as ps_dum,
        tc.tile_pool(name="ps_zt", bufs=2, space="PSUM") as ps_zt,
        tc.tile_pool(name="ps_tp", bufs=1, space="PSUM") as ps_tp,
        tc.tile_pool(name="ps_ext", bufs=2, space="PSUM") as ps_ext,
    ):
        ident = persist.tile([128, 128], F32)
        make_identity(nc, ident)
        ident_b = persist.tile([128, 128], BF16)
        nc.vector.tensor_copy(out=ident_b, in_=ident)

        # Persistent SBUF layouts (all matmul operands in bf16)
        # wr_t[p, kc, n] = W_rec[n, 128*kc + p]
        wr_t = persist.tile([128, NCH, STATE], BF16)
        # wo_t[p, nch, o] = W_out[o, 128*nch + p]
        wo_t = persist.tile([128, NCH, OUT], BF16)
        # wi_t[p, nch, n128] = W_in[128*nch + n128, p]
        wi_t = persist.tile([128, NCH, 128], BF16)
        sfin = persist.tile([128, 2, NCH // 2, BSH], BF16)  # parity-major chunks
        b_in_sb = persist.tile([128, NCH], F32)
        b_in_b = persist.tile([128, NCH], BF16)
        b_rec_nat = persist.tile([1, STATE], F32)
        bv_f = persist.tile([1, STATE], F32)    # biasv = b_rec + W_rec @ b_in
        bv_b = persist.tile([1, STATE], BF16)
        b_out_nat = persist.tile([1, OUT], F32)
        b_out_b = persist.tile([1, OUT], BF16)
        ones_f = persist.tile([1, BSH], F32)
        ones_b = persist.tile([1, BSH], BF16)
        osb = persist.tile([BSH, OUT], F32)
        nc.vector.memset(ones_f, 1.0)
        nc.vector.tensor_copy(out=ones_b, in_=ones_f)

        # ---- bias loads ----
        nc.sync.dma_start(out=b_in_sb, in_=b_in_d.rearrange("(q p) -> p q", p=128))
        nc.sync.dma_start(out=b_rec_nat, in_=b_rec_d.rearrange("(o n) -> o n", o=1))
        nc.sync.dma_start(out=b_out_nat, in_=b_out_d.rearrange("(o n) -> o n", o=1))
        nc.vector.tensor_copy(out=b_in_b, in_=b_in_sb)
        nc.vector.tensor_copy(out=b_out_b, in_=b_out_nat)

        # ---- W_in: load natural, PE-transpose into wi_t (bf16) ----
        for nch_ in range(NCH):
            winat = small.tile([128, IN], F32, name="winat")
            nc.sync.dma_start(out=winat, in_=w_in_d[128 * nch_:128 * nch_ + 128, :])
            tp = ps_tp.tile([128, 128], F32, name="tp")
            nc.tensor.transpose(tp, winat, ident)
            nc.vector.tensor_copy(out=wi_t[:, nch_, :], in_=tp)

        # ---- W_rec: load natural by n-chunk, PE-transpose into wr_t (bf16) ----
        for nr in range(NCH):
            wrnat = nat.tile([128, STATE], F32, name="wnat")
            nc.sync.dma_start(out=wrnat, in_=w_rec_d[128 * nr:128 * nr + 128, :])
            for kc in range(NCH):
                tp = ps_tp.tile([128, 128], F32, name="tp")
                nc.tensor.transpose(tp, wrnat[:, 128 * kc:128 * kc + 128], ident)
                if kc % 2 == 0:
                    nc.vector.tensor_copy(out=wr_t[:, kc, 128 * nr:128 * nr + 128], in_=tp)
                else:
                    nc.scalar.copy(out=wr_t[:, kc, 128 * nr:128 * nr + 128], in_=tp)

        # ---- biasv = b_rec + W_rec @ b_in  (absorbs per-step b_in add) ----
        for h in range(2):
            bvp = ps_ext.tile([128, TB, BSH], F32, name="ep")
            cp = bvp.rearrange("p a b -> p (a b)")
            for kc in range(NCH):
                nc.tensor.matmul(
                    cp[0:1, :],
                    b_in_b[:, kc:kc + 1],
                    wr_t[:, kc, 512 * h:512 * h + 512],
                    start=(kc == 0), stop=(kc == NCH - 1),
                )
            nc.vector.tensor_add(
                bv_f[:, 512 * h:512 * h + 512],
                b_rec_nat[:, 512 * h:512 * h + 512],
                cp[0:1, :],
            )
        nc.vector.tensor_copy(out=bv_b, in_=bv_f)

        # ---- ext block generator: ext for t in [tb*TB, (tb+1)*TB), bf16 ----
        ext_tiles = [None] * NTB

        def ext_block(tb):
            t0 = tb * TB
            xts = xts_p.tile([128, 4, 128], BF16, name="xts")
            for lo in range(4):
                xl = small.tile([128, IN], F32, name="xl")
                for tt in range(4):
                    t_ = t0 + 4 * lo + tt
                    nc.sync.dma_start(out=xl[32 * tt:32 * tt + 32, :], in_=x_d[:, t_, :])
                xtp = ps_tp.tile([128, 128], F32, name="tp")
                nc.tensor.transpose(xtp, xl, ident)
                nc.scalar.copy(out=xts[:, lo, :], in_=xtp)
                yield
            xts2 = xts.rearrange("p l c -> p (l c)")
            # eblk parity-major: eblk[:, t, m, q, :] holds ext chunk (2q+m)
            for nch_ in range(NCH):
                ep = ps_ext.tile([128, TB, BSH], F32, name="ep")
                epf = ep.rearrange("p a b -> p (a b)")
                # split the N=512 matmul so each bg item stays small
                nc.tensor.matmul(
                    epf[:, 0:256], wi_t[:, nch_, :], xts2[:, 0:256],
                    start=True, stop=True,
                )
                yield
                nc.tensor.matmul(
                    epf[:, 256:512], wi_t[:, nch_, :], xts2[:, 256:512],
                    start=True, stop=True,
                )
                if nch_ == 0:
                    eblk = extp.tile([128, TB, 2, NCH // 2, BSH], BF16, name="eblk")
                    ext_tiles[tb] = eblk
                nc.vector.tensor_copy(
                    out=eblk[:, 0:TB // 2, nch_ % 2, nch_ // 2, :],
                    in_=ep[:, 0:TB // 2, :],
                )
                yield
                nc.vector.tensor_copy(
                    out=eblk[:, TB // 2:TB, nch_ % 2, nch_ // 2, :],
                    in_=ep[:, TB // 2:TB, :],
                )
                yield

        def wout_chunk(oc):
            wonat = nat.tile([128, STATE], F32, name="wnat")
            nc.sync.dma_start(out=wonat, in_=w_out_d[128 * oc:128 * oc + 128, :])
            yield
            for nch_ in range(NCH):
                tp = ps_tp.tile([128, 128], F32, name="tp")
                nc.tensor.transpose(tp, wonat[:, 128 * nch_:128 * nch_ + 128], ident)
                nc.scalar.copy(out=wo_t[:, nch_, 128 * oc:128 * oc + 128], in_=tp)
                yield

        # block 0 fully before the recurrence
        for _ in ext_block(0):
            pass

        bg_blocks = [ext_block(tb) for tb in range(1, NTB)]
        bg_starts = [max(0, TB * (tb - RING) + TB - 1) for tb in range(1, NTB)]
        bg_idx = 0

        def wout_gen():
            for oc in range(NCH):
                yield from wout_chunk(oc)

        wout_it = wout_gen()

        def pop_bg(t, budget):
            nonlocal bg_idx
            while budget > 0:
                if bg_idx < len(bg_blocks) and t >= bg_starts[bg_idx]:
                    if next(bg_blocks[bg_idx], _DONE) is _DONE:
                        bg_idx += 1
                        continue
                    budget -= 1
                else:
                    if next(wout_it, _DONE) is _DONE:
                        break
                    budget -= 1

        # ---- recurrence ----
        # Step t: z = u_t @ W_rec.T + biasv   (4 column-tiled PE groups)
        #         u_{t+1} = relu(z) + ext_{t+1}
        # relu on ACT (PSUM->SBUF bf16), transpose back to state-layout on PE
        # (2x 128x128), ext add on DVE (bf16 2x mode).
        # State u_t lives as two tiles: evens/odds chunks from transposes m=0/1.
        st_chunks = [ext_tiles[0][:, 0, kc % 2, kc // 2, :] for kc in range(NCH)]
        zprev = None  # (z_sb, zt) of previous step

        def emit_bias(z):
            # bias init: 4 concurrent rank-1 MMs (ones x biasv); emitted right
            # after the previous step's k-MMs so the PE has work during the
            # relu/transpose chain.
            for g in range(NG):
                nc.tensor.matmul(
                    z[32 * g:32 * g + 32, :],
                    ones_b,
                    bv_b[:, GW * g:GW * g + GW],
                    start=True, stop=False,
                    tile_position=(0, 32 * g),
                )

        def emit_keepalive(n):
            # HAM keepalive: the PE activity monitor re-throttles the clock to
            # 1.2GHz unless the PE stays busy; these scratch matmuls (never
            # read) fill the relu/transpose dependency window each step.
            for _ in range(n):
                dm = ps_dum.tile([BSH, 256], F32, name="dum")
                nc.tensor.matmul(
                    dm, ident_b[:, 0:BSH], wr_t[:, 0, 0:256],
                    start=True, stop=True,
                )

        z = ps_z.tile([128, GW], F32, name="z")
        emit_bias(z)
        for t in range(T + 1):
            if t > 0:
                z_sb_p, zt_p = zprev
                # PE transposes of relu'd z back to state layout
                for m in range(2):
                    nc.tensor.transpose(
                        zt_p[:, m, :], z_sb_p[:, 128 * m:128 * m + 128], ident_b
                    )
                if t < T:
                    tb2, lt = t // TB, t % TB
                    assert tb2 == 0 or bg_idx > tb2 - 1, f"ext block {tb2} not emitted by step {t}"
                    stn_e = stp.tile([128, NCH // 2, BSH], BF16, name="stn_e")
                    stn_o = stp.tile([128, NCH // 2, BSH], BF16, name="stn_o")
                    ZT = zt_p.rearrange("p m (q b) -> p m q b", q=NG)
                    nc.vector.tensor_add(
                        stn_e, ZT[:, 0, :, :], ext_tiles[tb2][:, lt, 0, :, :]
                    )
                    nc.vector.tensor_add(
                        stn_o, ZT[:, 1, :, :], ext_tiles[tb2][:, lt, 1, :, :]
                    )
                    st_chunks = [
                        (stn_e if kc % 2 == 0 else stn_o)[:, kc // 2, :]
                        for kc in range(NCH)
                    ]
                else:
                    # final state: no ext add (sfin parity-major)
                    ZT = zt_p.rearrange("p m (q b) -> p m q b", q=NG)
                    nc.vector.tensor_copy(out=sfin[:, 0, :, :], in_=ZT[:, 0, :, :])
                    nc.vector.tensor_copy(out=sfin[:, 1, :, :], in_=ZT[:, 1, :, :])
            if t < T:
                for kc in range(NCH):
                    for g in range(NG):
                        nc.tensor.matmul(
                            z[32 * g:32 * g + 32, :],
                            st_chunks[kc],
                            wr_t[:, kc, GW * g:GW * g + GW],
                            start=False, stop=(kc == NCH - 1),
                            tile_position=(0, 32 * g),
                        )
                # relu + cast to bf16: half 0 on DVE, half 1 on ACT (parallel)
                z_sb = zsbp.tile([128, 256], BF16, name="z_sb")
                nc.vector.tensor_relu(z_sb[:, 0:128], z[:, 0:128])
                nc.scalar.activation(z_sb[:, 128:256], z[:, 128:256], RELU)
                zt = ps_zt.tile([128, 2, 128], BF16, name="zt")
                zprev = (z_sb, zt)
                if t < T - 1:
                    zn = ps_z.tile([128, GW], F32, name="z")
                    emit_bias(zn)
                    z = zn
            # background PE work after bias, still inside the post-chain window
            pop_bg(t, BG_PER_STEP)
            if 0 < t < T:
                emit_keepalive(2)

        assert bg_idx == len(bg_blocks), "ext blocks not fully emitted"
        for _ in wout_it:
            pass

        # ---- readout: out = sfin @ W_out.T + b_out ----
        for h in range(2):
            rop = ps_ext.tile([128, TB, BSH], F32, name="ep")
            ro = rop.rearrange("p a b -> p (a b)")[0:BSH, :]
            nc.tensor.matmul(
                ro, ones_b, b_out_b[:, 512 * h:512 * h + 512],
                start=True, stop=False,
            )
            for nch_ in range(NCH):
                nc.tensor.matmul(
                    ro, sfin[:, nch_ % 2, nch_ // 2, :],
                    wo_t[:, nch_, 512 * h:512 * h + 512],
                    start=False, stop=(nch_ == NCH - 1),
                )
            nc.vector.tensor_copy(out=osb[:, 512 * h:512 * h + 512], in_=ro)
        nc.sync.dma_start(out=out_d[:, :], in_=osb)


def build_nc():
    nc = bacc.Bacc(None, target_bir_lowering=False)
    x_d = nc.dram_tensor("x", [BSH, T, IN], F32, kind="ExternalInput")
    w_in_d = nc.dram_tensor("W_in", [STATE, IN], F32, kind="ExternalInput")
    b_in_d = nc.dram_tensor("b_in", [STATE], F32, kind="ExternalInput")
    w_rec_d = nc.dram_tensor("W_rec", [STATE, STATE], F32, kind="ExternalInput")
    b_rec_d = nc.dram_tensor("b_rec", [STATE], F32, kind="ExternalInput")
    w_out_d = nc.dram_tensor("W_out", [OUT, STATE], F32, kind="ExternalInput")
    b_out_d = nc.dram_tensor("b_out", [OUT], F32, kind="ExternalInput")
    out_d = nc.dram_tensor("out", [BSH, OUT], F32, kind="ExternalOutput")
    with tile.TileContext(nc) as tc:
        _build(tc, x_d, w_in_d, b_in_d, w_rec_d, b_rec_d, w_out_d, b_out_d, out_d)
    return nc


def kernel(**inputs):
    global LAST_EXEC_NS, LAST_RESULTS
    nc = build_nc()
    nc.finalize()

    def f32c(a):
        return np.ascontiguousarray(np.asarray(a, dtype=np.float32))

    shared = {k: f32c(inputs[k]) for k in ("W_in", "b_in", "W_rec", "b_rec", "W_out", "b_out")}
    x = f32c(inputs["x"])
    in_maps = []
    for c in range(NCORES):
        m = dict(shared)
        m["x"] = np.ascontiguousarray(x[c * BSH:(c + 1) * BSH])
        in_maps.append(m)

    res = run_bass_kernel_spmd(nc, in_maps, list(range(NCORES)), trace=TRACE)
    LAST_EXEC_NS = res.exec_time_ns
    LAST_RESULTS = res
    plop = np.concatenate([res.results[c]["out"] for c in range(NCORES)], axis=0)
    return np.ascontiguousarray(
        np.broadcast_to(plop[:, None, :], (BS, T, OUT)).astype(np.float32)
    )


# revision 18
# speedup vs baseline: 1.0069x; 1.0069x over previous
import sys

import numpy as np

sys.path.insert(0, "/opt/trn_rl_repo")

import concourse.bacc as bacc
import concourse.tile as tile
from concourse import mybir
from concourse.bass_utils import run_bass_kernel_spmd
from concourse.masks import make_identity

BS, T, IN, STATE, OUT = 256, 128, 128, 1024, 1024
NCORES = 8
BSH = BS // NCORES  # 32 batch rows per core
NCH = STATE // 128  # 8 state chunks of 128
TB = 16             # timesteps per ext block
NTB = T // TB       # 8
RING = 3            # ext ring depth (blocks resident)
NG = 4              # PE column-tile groups for the recurrence matmul
GW = STATE // NG    # 256 output state cols per group

TRACE = False
BG_PER_STEP = 2

LAST_EXEC_NS = None
LAST_RESULTS = None
_DONE = object()

F32 = mybir.dt.float32
BF16 = mybir.dt.bfloat16
RELU = mybir.ActivationFunctionType.Relu


def _build(tc, x_d, w_in_d, b_in_d, w_rec_d, b_rec_d, w_out_d, b_out_d, out_d):
    nc = tc.nc

    with (
        tc.tile_pool(name="persist", bufs=1) as persist,
        tc.tile_pool(name="extp", bufs=RING) as extp,
        tc.tile_pool(name="nat", bufs=2) as nat,
        tc.tile_pool(name="small", bufs=2) as small,
        tc.tile_pool(name="xts_p", bufs=2) as xts_p,
        tc.tile_pool(name="st", bufs=2) as stp,
        tc.tile_pool(name="zsb", bufs=2) as zsbp,
        tc.tile_pool(name="ps_z", bufs=2, space="PSUM") as ps_z,
        tc.tile_pool(name="ps_dum", bufs=1, space="PSUM") as ps_dum,
        tc.tile_pool(name="ps_zt", bufs=2, space="PSUM") as ps_zt,
        tc.tile_pool(name="ps_tp", bufs=1, space="PSUM") as ps_tp,
        tc.tile_pool(name="ps_ext", bufs=2, space="PSUM") as ps_ext,
    ):
        ident = persist.tile([128, 128], F32)
        make_identity(nc, ident)
        ident_b = persist.tile([128, 128], BF16)
        nc.vector.tensor_copy(out=ident_b, in_=ident)

        # Persistent SBUF layouts (all matmul operands in bf16)
        # wr_t[p, kc, n] = W_rec[n, 128*kc + p]
        wr_t = persist.tile([128, NCH, STATE], BF16)
        # wo_t[p, nch, o] = W_out[o, 128*nch + p]
        wo_t = persist.tile([128, NCH, OUT], BF16)
        # wi_t[p, nch, n128] = W_in[128*nch + n128, p]
        wi_t = persist.tile([128, NCH, 128], BF16)
        sfin = persist.tile([128, 2, NCH // 2, BSH], BF16)  # parity-major chunks
        b_in_sb = persist.tile([128, NCH], F32)
        b_in_b = persist.tile([128, NCH], BF16)
        b_rec_nat = persist.tile([1, STATE], F32)
        bv_f = persist.tile([1, STATE], F32)    # biasv = b_rec + W_rec @ b_in
        bv_b = persist.tile([1, STATE], BF16)
        b_out_nat = persist.tile([1, OUT], F32)
        b_out_b = persist.tile([1, OUT], BF16)
        ones_f = persist.tile([1, BSH], F32)
        ones_b = persist.tile([1, BSH], BF16)
        osb = persist.tile([BSH, OUT], F32)
        nc.vector.memset(ones_f, 1.0)
        nc.vector.tensor_copy(out=ones_b, in_=ones_f)

        # ---- bias loads ----
        nc.sync.dma_start(out=b_in_sb, in_=b_in_d.rearrange("(q p) -> p q", p=128))
        nc.sync.dma_start(out=b_rec_nat, in_=b_rec_d.rearrange("(o n) -> o n", o=1))
        nc.sync.dma_start(out=b_out_nat, in_=b_out_d.rearrange("(o n) -> o n", o=1))
        nc.vector.tensor_copy(out=b_in_b, in_=b_in_sb)
        nc.vector.tensor_copy(out=b_out_b, in_=b_out_nat)

        # ---- W_in: load natural, PE-transpose into wi_t (bf16) ----
        for nch_ in range(NCH):
            winat = small.tile([128, IN], F32, name="winat")
            nc.sync.dma_start(out=winat, in_=w_in_d[128 * nch_:128 * nch_ + 128, :])
            tp = ps_tp.tile([128, 128], F32, name="tp")
            nc.tensor.transpose(tp, winat, ident)
            nc.vector.tensor_copy(out=wi_t[:, nch_, :], in_=tp)

        # ---- W_rec: load natural by n-chunk, PE-transpose into wr_t (bf16) ----
        for nr in range(NCH):
            wrnat = nat.tile([128, STATE], F32, name="wnat")
            nc.sync.dma_start(out=wrnat, in_=w_rec_d[128 * nr:128 * nr + 128, :])
            for kc in range(NCH):
                tp = ps_tp.tile([128, 128], F32, name="tp")
                nc.tensor.transpose(tp, wrnat[:, 128 * kc:128 * kc + 128], ident)
                if kc % 2 == 0:
                    nc.vector.tensor_copy(out=wr_t[:, kc, 128 * nr:128 * nr + 128], in_=tp)
                else:
                    nc.scalar.copy(out=wr_t[:, kc, 128 * nr:128 * nr + 128], in_=tp)

        # ---- biasv = b_rec + W_rec @ b_in  (absorbs per-step b_in add) ----
        for h in range(2):
            bvp = ps_ext.tile([128, TB, BSH], F32, name="ep")
            cp = bvp.rearrange("p a b -> p (a b)")
            for kc in range(NCH):
                nc.tensor.matmul(
                    cp[0:1, :],
                    b_in_b[:, kc:kc + 1],
                    wr_t[:, kc, 512 * h:512 * h + 512],
                    start=(kc == 0), stop=(kc == NCH - 1),
                )
            nc.vector.tensor_add(
                bv_f[:, 512 * h:512 * h + 512],
                b_rec_nat[:, 512 * h:512 * h + 512],
                cp[0:1, :],
            )
        nc.vector.tensor_copy(out=bv_b, in_=bv_f)

        # ---- ext block generator: ext for t in [tb*TB, (tb+1)*TB), bf16 ----
        ext_tiles = [None] * NTB

        def ext_block(tb):
            t0 = tb * TB
            xts = xts_p.tile([128, 4, 128], BF16, name="xts")
            for lo in range(4):
                xl = small.tile([128, IN], F32, name="xl")
                for tt in range(4):
                    t_ = t0 + 4 * lo + tt
                    nc.sync.dma_start(out=xl[32 * tt:32 * tt + 32, :], in_=x_d[:, t_, :])
                xtp = ps_tp.tile([128, 128], F32, name="tp")
                nc.tensor.transpose(xtp, xl, ident)
                nc.scalar.copy(out=xts[:, lo, :], in_=xtp)
                yield
            xts2 = xts.rearrange("p l c -> p (l c)")
            # eblk parity-major: eblk[:, t, m, q, :] holds ext chunk (2q+m)
            for nch_ in range(NCH):
                ep = ps_ext.tile([128, TB, BSH], F32, name="ep")
                epf = ep.rearrange("p a b -> p (a b)")
                # split the N=512 matmul so each bg item stays small
                nc.tensor.matmul(
                    epf[:, 0:256], wi_t[:, nch_, :], xts2[:, 0:256],
                    start=True, stop=True,
                )
                yield
                nc.tensor.matmul(
                    epf[:, 256:512], wi_t[:, nch_, :], xts2[:, 256:512],
                    start=True, stop=True,
                )
                if nch_ == 0:
                    eblk = extp.tile([128, TB, 2, NCH // 2, BSH], BF16, name="eblk")
                    ext_tiles[tb] = eblk
                nc.vector.tensor_copy(
                    out=eblk[:, 0:TB // 2, nch_ % 2, nch_ // 2, :],
                    in_=ep[:, 0:TB // 2, :],
                )
                yield
                nc.vector.tensor_copy(
                    out=eblk[:, TB // 2:TB, nch_ % 2, nch_ // 2, :],
                    in_=ep[:, TB // 2:TB, :],
                )
                yield

        def wout_chunk(oc):
            wonat = nat.tile([128, STATE], F32, name="wnat")
            nc.sync.dma_start(out=wonat, in_=w_out_d[128 * oc:128 * oc + 128, :])
            yield
            for nch_ in range(NCH):
                tp = ps_tp.tile([128, 128], F32, name="tp")
                nc.tensor.transpose(tp, wonat[:, 128 * nch_:128 * nch_ + 128], ident)
                nc.scalar.copy(out=wo_t[:, nch_, 128 * oc:128 * oc + 128], in_=tp)
                yield

        # block 0 fully before the recurrence
        for _ in ext_block(0):
            pass

        bg_blocks = [ext_block(tb) for tb in range(1, NTB)]
        bg_starts = [max(0, TB * (tb - RING) + TB - 1) for tb in range(1, NTB)]
        bg_idx = 0

        def wout_gen():
            for oc in range(NCH):
                yield from wout_chunk(oc)

        wout_it = wout_gen()

        def pop_bg(t, budget):
            nonlocal bg_idx
            while budget > 0:
                if bg_idx < len(bg_blocks) and t >= bg_starts[bg_idx]:
                    if next(bg_blocks[bg_idx], _DONE) is _DONE:
                        bg_idx += 1
                        continue
                    budget -= 1
                else:
                    if next(wout_it, _DONE) is _DONE:
                        break
                    budget -= 1

        # ---- recurrence ----
        # Step t: z = u_t @ W_rec.T + biasv   (4 column-tiled PE groups)
        #         u_{t+1} = relu(z) + ext_{t+1}
        # relu on ACT (PSUM->SBUF bf16), transpose back to state-layout on PE
        # (2x 128x128), ext add on DVE (bf16 2x mode).
        # State u_t lives as two tiles: evens/odds chunks from transposes m=0/1.
        st_chunks = [ext_tiles[0][:, 0, kc % 2, kc // 2, :] for kc in range(NCH)]
        zprev = None  # (z_sb, zt) of previous step

        def emit_bias(z):
            # bias init: 4 concurrent rank-1 MMs (ones x biasv); emitted right
            # after the previous step's k-MMs so the PE has work during the
            # relu/transpose chain.
            for g in range(NG):
                nc.tensor.matmul(
                    z[32 * g:32 * g + 32, :],
                    ones_b,
                    bv_b[:, GW * g:GW * g + GW],
                    start=True, stop=False,
                    tile_position=(0, 32 * g),
                )

        def emit_keepalive(n):
            # HAM keepalive: the PE activity monitor re-throttles the clock to
            # 1.2GHz unless the PE stays busy; these scratch matmuls (never
            # read) fill the relu/transpose dependency window each step.
            for _ in range(n):
                dm = ps_dum.tile([BSH, 256], F32, name="dum")
                nc.tensor.matmul(
                    dm, ident_b[:, 0:BSH], wr_t[:, 0, 0:256],
                    start=True, stop=True,
                )

        z = ps_z.tile([128, GW], F32, name="z")
        emit_bias(z)
        for t in range(T + 1):
            if t > 0:
                z_sb_p, zt_p = zprev
                # PE transposes of relu'd z back to state layout
                for m in range(2):
                    nc.tensor.transpose(
                        zt_p[:, m, :], z_sb_p[:, 128 * m:128 * m + 128], ident_b
                    )
                if t < T:
                    tb2, lt = t // TB, t % TB
                    assert tb2 == 0 or bg_idx > tb2 - 1, f"ext block {tb2} not emitted by step {t}"
                    stn_e = stp.tile([128, NCH // 2, BSH], BF16, name="stn_e")
                    stn_o = stp.tile([128, NCH // 2, BSH], BF16, name="stn_o")
                    ZT = zt_p.rearrange("p m (q b) -> p m q b", q=NG)
                    nc.vector.tensor_add(
                        stn_e, ZT[:, 0, :, :], ext_tiles[tb2][:, lt, 0, :, :]
                    )
                    nc.vector.tensor_add(
                        stn_o, ZT[:, 1, :, :], ext_tiles[tb2][:, lt, 1, :, :]
                    )
                    st_chunks = [
                        (stn_e if kc % 2 == 0 else stn_o)[:, kc // 2, :]
                        for kc in range(NCH)
                    ]
                    emit_keepalive(2)
                else:
                    # final state: no ext add (sfin parity-major)
                    ZT = zt_p.rearrange("p m (q b) -> p m q b", q=NG)
                    nc.vector.tensor_copy(out=sfin[:, 0, :, :], in_=ZT[:, 0, :, :])
                    nc.vector.tensor_copy(out=sfin[:, 1, :, :], in_=ZT[:, 1, :, :])
            if t < T:
                for kc in range(NCH):
                    for g in range(NG):
                        nc.tensor.matmul(
                            z[32 * g:32 * g + 32, :],
                            st_chunks[kc],
                            wr_t[:, kc, GW * g:GW * g + GW],
                            start=False, stop=(kc == NCH - 1),
                            tile_position=(0, 32 * g),
                        )
                # relu + cast to bf16: half 0 on DVE, half 1 on ACT (parallel)
                z_sb = zsbp.tile([128, 256], BF16, name="z_sb")
                nc.vector.tensor_relu(z_sb[:, 0:128], z[:, 0:128])
                nc.scalar.activation(z_sb[:, 128:256], z[:, 128:256], RELU)
                zt = ps_zt.tile([128, 2, 128], BF16, name="zt")
                zprev = (z_sb, zt)
                if t < T - 1:
                    zn = ps_z.tile([128, GW], F32, name="z")
                    emit_bias(zn)
                    z = zn
            # background PE work after bias, still inside the post-chain window
            pop_bg(t, BG_PER_STEP)

        assert bg_idx == len(bg_blocks), "ext blocks not fully emitted"
        for _ in wout_it:
            pass

        # ---- readout: out = sfin @ W_out.T + b_out ----
        for h in range(2):
            rop = ps_ext.tile([128, TB, BSH], F32, name="ep")
            ro = rop.rearrange("p a b -> p (a b)")[0:BSH, :]
            nc.tensor.matmul(
                ro, ones_b, b_out_b[:, 512 * h:512 * h + 512],
                start=True, stop=False,
            )
            for nch_ in range(NCH):
                nc.tensor.matmul(
                    ro, sfin[:, nch_ % 2, nch_ // 2, :],
                    wo_t[:, nch_, 512 * h:512 * h + 512],
                    start=False, stop=(nch_ == NCH - 1),
                )
            nc.vector.tensor_copy(out=osb[:, 512 * h:512 * h + 512], in_=ro)
        nc.sync.dma_start(out=out_d[:, :], in_=osb)


def build_nc():
    nc = bacc.Bacc(None, target_bir_lowering=False)
    x_d = nc.dram_tensor("x", [BSH, T, IN], F32, kind="ExternalInput")
    w_in_d = nc.dram_tensor("W_in", [STATE, IN], F32, kind="ExternalInput")
    b_in_d = nc.dram_tensor("b_in", [STATE], F32, kind="ExternalInput")
    w_rec_d = nc.dram_tensor("W_rec", [STATE, STATE], F32, kind="ExternalInput")
    b_rec_d = nc.dram_tensor("b_rec", [STATE], F32, kind="ExternalInput")
    w_out_d = nc.dram_tensor("W_out", [OUT, STATE], F32, kind="ExternalInput")
    b_out_d = nc.dram_tensor("b_out", [OUT], F32, kind="ExternalInput")
    out_d = nc.dram_tensor("out", [BSH, OUT], F32, kind="ExternalOutput")
    with tile.TileContext(nc) as tc:
        _build(tc, x_d, w_in_d, b_in_d, w_rec_d, b_rec_d, w_out_d, b_out_d, out_d)
    return nc


def kernel(**inputs):
    global LAST_EXEC_NS, LAST_RESULTS
    nc = build_nc()
    nc.finalize()

    def f32c(a):
        return np.ascontiguousarray(np.asarray(a, dtype=np.float32))

    shared = {k: f32c(inputs[k]) for k in ("W_in", "b_in", "W_rec", "b_rec", "W_out", "b_out")}
    x = f32c(inputs["x"])
    in_maps = []
    for c in range(NCORES):
        m = dict(shared)
        m["x"] = np.ascontiguousarray(x[c * BSH:(c + 1) * BSH])
        in_maps.append(m)

    res = run_bass_kernel_spmd(nc, in_maps, list(range(NCORES)), trace=TRACE)
    LAST_EXEC_NS = res.exec_time_ns
    LAST_RESULTS = res
    plop = np.concatenate([res.results[c]["out"] for c in range(NCORES)], axis=0)
    return np.ascontiguousarray(
        np.broadcast_to(plop[:, None, :], (BS, T, OUT)).astype(np.float32)
    )
